# revision 38
# baseline (speedup 1.0000x reference)
"""GATedge kernel for Trainium2, 8 NeuronCores, batch-parallel (1 batch elem / core).

v2: o-on-partitions layout - transpose-free.

Math (per batch b), restructured from the reference:
  s_e  = dot(W_edge, attn_l); wl = W_src @ attn_l; wr = W_dst @ attn_r  (host)
  score[o,m] = el[o] + er[m] + efm[o,m]   (adj multiply dropped: masked
    slots carry -1e30 in efm so their pre-mask value is irrelevant)
  p    = exp(max(score, 0.2*score))       (lrelu via max)
  pk   = exp(lrelu(2*er));  Z[m] = sum_o p + pk
  F^T  = sum_c fs_c^T @ p_c  (fs = h_src @ W_src)   [PE, o-contraction]
  S1   = g^T @ 1, g = p * efm;  Zcol = p^T @ 1      [PE ones-matmuls]
  out  = sigmoid((S1*W_edge/s_e + F + pk*feat_dst) / Z)

Layout: o padded to 1024 = 8 chunks x 128 partitions. Score tensors are
(128, 512): block c cols [64c,64c+64) hold o-chunk c (partition p = o-128c).
el enters the score PSUM via matmuls with lhsT=hsT_c, rhs=wl tiled 64 wide
(broadcast across m); er via lhsT=wr tiled 128 wide, rhs=hdT (broadcast
across partitions). Pad rows carry efm=-1e30 so p=0 there.
F^T (d-part, m-free) needs ONE transpose back to m-part for the combine.

All large tensors bf16 (rel err 1.3e-2 vs the 2e-2 gate); PSUM fp32.

Hardware rules learned the hard way (violations crash the exec unit):
 - never read PSUM rows a matmul didn't write
 - a PE transpose must be followed by its PSUM->SBUF copy before the next
   transpose into the same bank
 - tensor_tensor with both SBUF inputs needs equal base partitions
 - ACT Lrelu gives wrong results (alpha semantics) - use max(x, 0.2x)
"""

import sys

for _p in ("/opt/trn_rl_repo", "/root/.axon_site/_ro/trn_rl_repo"):
    if _p not in sys.path:
        sys.path.insert(0, _p)

import numpy as np
import ml_dtypes

BF16 = ml_dtypes.bfloat16

B, NO, NM, KS, KD, D = 8, 1000, 64, 128, 64, 128
NOP = 1024                # o padded
NC = 8                    # o-chunks
SW = NC * NM              # score width = 512
N_CORES = 8
NEG_SLOPE = 0.2
BIG = 1e30

# packed param buffer column layout (par0 = cols [0, P_WD); par1 = rest)
P_WL, P_WS, P_WR, P_HD, P_WC, P_ON, P_ID, P_OR = 0, 64, 192, 320, 384, 385, 386, 514
P_WD, P_WEB = 642, 770
P_COLS = 898

_cache = {}


def _build():
    import os
    import concourse.tile as tile
    from concourse import bacc, mybir
    from concourse.tile import add_dep_helper

    stage = int(os.environ.get("BASS_GAT_STAGE", "9"))
    f32 = mybir.dt.float32
    bf = mybir.dt.bfloat16
    AF = mybir.ActivationFunctionType
    ALU = mybir.AluOpType

    nc = bacc.Bacc("TRN2", target_bir_lowering=False, debug=False,
                   num_devices=N_CORES)

    d_hsT = nc.dram_tensor("hsT", [KS, NOP], bf, kind="ExternalInput")
    d_ea = nc.dram_tensor("ea", [128, SW], bf, kind="ExternalInput")
    d_par = nc.dram_tensor("par", [128, P_COLS], bf, kind="ExternalInput")
    d_out = nc.dram_tensor("out", [NM, D], f32, kind="ExternalOutput")

    def _emit(tc, sb, ps):
        # ---- input DMAs: par0(SP), hsT0(Pool swdge), ea0(ACT), hsT1(SP),
        # ea1(Pool swdge), par1(SP) ----
        s_par = sb.tile([128, P_COLS], bf, tag="par")
        nc.sync.dma_start(s_par[:, 0:P_WD], d_par[:, 0:P_WD])
        s_hsT = sb.tile([KS, NOP], bf, tag="hsT")
        nc.gpsimd.dma_start(s_hsT[:, 0:512], d_hsT[:, 0:512])
        s_ea = sb.tile([128, SW], bf, tag="ea")
        nc.scalar.dma_start(s_ea[:, 0:256], d_ea[:, 0:256])
        nc.sync.dma_start(s_hsT[:, 512:NOP], d_hsT[:, 512:NOP])
        nc.gpsimd.dma_start(s_ea[:, 256:SW], d_ea[:, 256:SW])
        nc.sync.dma_start(s_par[:, P_WD:P_COLS], d_par[:, P_WD:P_COLS])

        wl64 = s_par[:, P_WL:P_WL + NM]
        wsrc = s_par[:, P_WS:P_WS + D]
        wr128 = s_par[0:KD, P_WR:P_WR + D]
        hdT = s_par[0:KD, P_HD:P_HD + NM]
        wrcol = s_par[0:KD, P_WC:P_WC + 1]
        s_ones = s_par[:, P_ON:P_ON + 1]
        s_ident = s_par[:, P_ID:P_ID + D]   # host-provided 128x128 identity
        ones_row = s_par[0:1, P_OR:P_OR + D]
        wdst = s_par[0:KD, P_WD:P_WD + D]
        web128 = s_par[:, P_WEB:P_WEB + D]

        # ---- PE: er column for ekk; per chunk er+el accumulate pair
        # (er start=True must be immediately followed by its el start=False:
        # an intervening start=True on the same PSUM bank resets the group).
        # score/fs psum split per half: Tile deps are tile-granular, so one
        # big tile would gate chunk-0 consumers on chunk-7 producers.
        # ps_small regions: col0 er_col, col1 Z_col, cols 4:68 er_row (p0),
        # cols 68:132 S1_row (p0). Groups stay sequential within this bank.
        ps_small = ps.tile([KD, 132], f32, tag="small", padded_shape=[KD, 512])
        nc.tensor.matmul(ps_small[:, 0:1], hdT, wrcol, start=True, stop=True)
        nc.tensor.matmul(ps_small[0:1, 4:68], wrcol, hdT,
                         start=True, stop=True)

        # per block: [er, el, ea] accumulate triple (must stay adjacent: a
        # later start=True in the same PSUM bank resets open groups), ea via
        # identity-matmul so the full score lands in PSUM with no DVE add.
        ps_score = [ps.tile([128, SW // 2], f32, tag=f"score{i}",
                            name=f"ps_score{i}") for i in range(2)]
        ps_fs = [ps.tile([128, NC * D // 2], f32, tag=f"fs{i}",
                         name=f"ps_fs{i}") for i in range(2)]
        for i in range(2):
            for cc in range(4):
                c = 4 * i + cc
                h_c = s_hsT[:, D * c:D * c + D]
                blk = ps_score[i][:, NM * cc:NM * cc + NM]
                nc.tensor.matmul(blk, wr128, hdT, start=True, stop=False)
                nc.tensor.matmul(blk, h_c, wl64, start=False, stop=False)
                nc.tensor.matmul(blk, s_ident, s_ea[:, NM * c:NM * c + NM],
                                 start=False, stop=True)
            for cc in range(4):
                c = 4 * i + cc
                h_c = s_hsT[:, D * c:D * c + D]
                nc.tensor.matmul(ps_fs[i][:, D * cc:D * cc + D], h_c, wsrc,
                                 start=True, stop=True)
        if stage <= 1:
            s_dbg = sb.tile([NM, D], f32, tag="dbg")
            nc.vector.tensor_copy(s_dbg[:], ps_score[0][0:NM, 0:D])
            nc.sync.dma_start(d_out[:], s_dbg[:])
            return

        # ---- ekk/pk prologue, both orientations - off critical path ----
        # column form (for Z fold): 2*er -> lrelu(on Pool) -> exp
        s_er2 = sb.tile([64, 1], f32, tag="er2")
        nc.scalar.mul(s_er2[:], ps_small[:, 0:1], 2.0)
        s_er2s = sb.tile([64, 1], f32, tag="er2s")
        nc.vector.tensor_scalar_mul(s_er2s[:], s_er2[:], NEG_SLOPE)
        s_ekk = sb.tile([64, 1], f32, tag="ekk")
        r_ekk = nc.vector.tensor_max(s_ekk[:], s_er2[:], s_er2s[:])
        s_pk = sb.tile([64, 1], f32, tag="pk")
        r_pk = nc.scalar.activation(s_pk[:], s_ekk[:], AF.Exp)
        # row form (for the pk*fd^T term): prelu(2*er_row) -> exp -> bf16
        s_ekr = sb.tile([1, NM], f32, tag="ekr")
        nc.scalar.activation(s_ekr[:], ps_small[0:1, 4:68], AF.Prelu,
                             scale=2.0, alpha=NEG_SLOPE)
        s_pkr = sb.tile([1, NM], bf, tag="pkr")
        r_pkr = nc.scalar.activation(s_pkr[:], s_ekr[:], AF.Exp)
        # pk broadcast to 64 partitions, then scale hdT columns by pk
        ps_pkb = ps.tile([KD, NM], f32, tag="pkb", padded_shape=[KD, 512])
        nc.tensor.matmul(ps_pkb[:], ones_row[0:1, 0:KD], s_pkr[:],
                         start=True, stop=True)
        s_hdTs = sb.tile([KD, NM], bf, tag="hdTs")
        r_hdTs = nc.vector.tensor_tensor(s_hdTs[:], hdT, ps_pkb[:], ALU.mult)

        # ---- score chain, 2 column chunks of 256: lrelu then exp.
        # prelu=1: single ACT Prelu from PSUM; else DVE mul+max fallback.
        prelu = os.environ.get("BASS_GAT_PRELU", "1") == "1"
        s_fs = sb.tile([128, NC * D], bf, tag="fs_sb")
        s_ay = sb.tile([128, SW], bf, tag="ay")
        s_eijk = sb.tile([128, SW], bf, tag="eijk")
        s_p = sb.tile([128, SW], bf, tag="p")
        r_exp = [None, None]
        r_fs = [None, None]
        r_last = None
        for i in range(2):
            c = slice(256 * i, 256 * i + 256)
            if prelu:
                prev = r_last
                r_last = nc.scalar.activation(s_eijk[:, c], ps_score[i][:],
                                              AF.Prelu, alpha=NEG_SLOPE)
                if i == 0:
                    # keep the tiny pk/pkr ACT ops ahead of the score chain
                    add_dep_helper(r_last.ins, r_pk.ins,
                                   reason="pk before score chain on ACT")
                    add_dep_helper(r_last.ins, r_pkr.ins,
                                   reason="pkr before score chain on ACT")
                else:
                    add_dep_helper(r_last.ins, prev.ins,
                                   reason="score chunk order on ACT")
            else:
                r_a = nc.vector.tensor_scalar_mul(s_ay[:, c], ps_score[i][:],
                                                  NEG_SLOPE)
                if r_last is not None:
                    add_dep_helper(r_a.ins, r_last.ins,
                                   reason="score chain chunk order on DVE")
                r_last = nc.vector.tensor_max(s_eijk[:, c], ps_score[i][:],
                                              s_ay[:, c])
            r_exp[i] = nc.scalar.activation(s_p[:, c], s_eijk[:, c], AF.Exp)
            # fs copies on DVE so the ACT queue stays clear for prelu/exp
            r_fs[i] = nc.vector.tensor_copy(s_fs[:, 512 * i:512 * i + 512],
                                            ps_fs[i][:])
        # DVE order: er2s/ekk, fs_h1, hdTs, g_0, fs_h2, g_1
        add_dep_helper(r_fs[0].ins, r_ekk.ins, reason="ekk before fs_h1")
        add_dep_helper(r_hdTs.ins, r_fs[0].ins, reason="hdTs after fs_h1")

        def after_chain(r):
            add_dep_helper(r.ins, r_last.ins,
                           reason="defer small DVE op past score chain")
            return r

        if stage <= 3:
            s_dbg = sb.tile([NM, D], f32, tag="dbg")
            nc.vector.tensor_copy(s_dbg[:], s_p[0:NM, 0:D])
            nc.sync.dma_start(d_out[:], s_dbg[:])
            return

        # g = p * efm (for S1), per chunk on DVE right after each exp
        s_g = sb.tile([128, SW], bf, tag="g")
        r_g = [None, None]
        for i in range(2):
            c = slice(256 * i, 256 * i + 256)
            r_g[i] = nc.vector.tensor_tensor(s_g[:, c], s_p[:, c],
                                             s_ea[:, c], ALU.mult)
        add_dep_helper(r_g[0].ins, r_hdTs.ins, reason="g_0 after hdTs")
        add_dep_helper(r_fs[1].ins, r_g[0].ins, reason="fs_h2 after g_0")
        add_dep_helper(r_g[1].ins, r_fs[1].ins, reason="g_1 after fs_h2")

        # ---- PE: the whole pre-sigmoid combine accumulates in ONE psum
        # tile tT (d-part, m-free): pk*fd^T (start) + F^T + web (x) S1 via
        # web128 x g_c matmuls (stop on the last). Z runs in ps_small after.
        ps_tT = ps.tile([D, NM], f32, tag="tT", padded_shape=[D, 512])
        nc.tensor.matmul(ps_tT[:], wdst, s_hdTs[:], start=True, stop=False)
        for c in range(NC):
            nc.tensor.matmul(ps_tT[:], s_fs[:, D * c:D * c + D],
                             s_p[:, NM * c:NM * c + NM],
                             start=False, stop=False)
        for c in range(NC):
            nc.tensor.matmul(ps_tT[:], web128, s_g[:, NM * c:NM * c + NM],
                             start=False, stop=(c == NC - 1))
        for c in range(NC):
            nc.tensor.matmul(ps_small[:, 1:2], s_p[:, NM * c:NM * c + NM],
                             s_ones[:], start=(c == 0), stop=(c == NC - 1))

        # Z = Zcol + pk; hZr = 0.5/Z (tanh scale)
        s_Z = sb.tile([64, 1], f32, tag="Z")
        after_chain(nc.vector.tensor_tensor(s_Z[:], ps_small[:, 1:2],
                                            s_pk[:], ALU.add))
        s_Zr = sb.tile([64, 1], f32, tag="Zr")
        after_chain(nc.vector.reciprocal(s_Zr[:], s_Z[:]))
        s_hZr = sb.tile([64, 1], f32, tag="hZr")
        after_chain(nc.vector.tensor_scalar_mul(s_hZr[:], s_Zr[:], 0.5))

        if stage <= 4:
            s_dbg = sb.tile([NM, D], f32, tag="dbg")
            nc.vector.tensor_scalar(s_dbg[:], s_p[0:NM, 0:D], s_Z[:],
                                    None, ALU.mult)
            nc.sync.dma_start(d_out[:], s_dbg[:])
            return

        # ---- tT -> (m, d) via one transpose, then sigmoid via tanh:
        # sigmoid(x) = 0.5 + 0.5*tanh(x/2)  (Tanh shares the Exp table) ----
        s_tT = sb.tile([D, NM], bf, tag="tT_sb")
        after_chain(nc.vector.tensor_copy(s_tT[:], ps_tT[:]))
        ps_F = ps.tile([NM, D], bf, tag="F", padded_shape=[NM, 512])
        nc.tensor.transpose(ps_F[:], s_tT[:], s_ident[:])
        s_th = sb.tile([NM, D], f32, tag="th")
        nc.scalar.activation(s_th[:], ps_F[:], AF.Tanh, scale=s_hZr[:])
        s_out = sb.tile([NM, D], f32, tag="out_sb")
        after_chain(nc.vector.tensor_scalar(s_out[:], s_th[:], 0.5, 0.5,
                                            ALU.mult, ALU.add))
        nc.sync.dma_start(d_out[:], s_out[:])

    with tile.TileContext(nc) as tc:
        with tc.tile_pool(name="sb", bufs=1) as sb, \
             tc.tile_pool(name="ps", bufs=1, space="PSUM") as ps:
            _emit(tc, sb, ps)

    nc.compile()
    return nc


def _get_nc():
    if "nc" not in _cache:
        _cache["nc"] = _build()
    return _cache["nc"]


def _prep_core_inputs(h_src, h_dst, edge_feat, adj, W_src, W_dst, W_edge,
                      attn_l, attn_r):
    f32, f64 = np.float32, np.float64
    wl = (W_src.astype(f64) @ attn_l.astype(f64)).astype(f32)
    wr = (W_dst.astype(f64) @ attn_r.astype(f64)).astype(f32)
    s_e = float(np.dot(W_edge.astype(f64), attn_l.astype(f64)))
    s_e_safe = s_e if abs(s_e) > 1e-20 else 1e-20

    par = np.zeros((128, P_COLS), BF16)
    par[:, P_WL:P_WL + NM] = np.tile(wl[:, None], (1, NM))
    par[:, P_WS:P_WS + D] = W_src
    par[0:KD, P_WR:P_WR + D] = np.tile(wr[:, None], (1, D))
    par[0:KD, P_WC:P_WC + 1] = wr[:, None]
    par[:, P_ON:P_ON + 1] = 1.0
    par[:, P_ID:P_ID + D] = np.eye(128, dtype=f32)
    par[0:1, P_OR:P_OR + D] = 1.0
    par[0:KD, P_WD:P_WD + D] = W_dst
    par[:, P_WEB:P_WEB + D] = np.tile(
        (W_edge.astype(f64) / s_e_safe).astype(f32)[None, :], (128, 1))

    # fold the adjacency mask into the edge term: -1e30 where adj=0 makes
    # exp() zero those slots; pad rows (o >= 1000) also get -1e30
    ef_s = ((edge_feat.astype(f64) * s_e) +
            (adj.astype(f64) - 1.0) * BIG).astype(f32)  # (B, NO, NM)

    in_maps = []
    for b in range(B):
        parb = par.copy()
        parb[0:KD, P_HD:P_HD + NM] = h_dst[b].T
        efp = np.full((NOP, NM), -BIG, f32)
        efp[0:NO] = ef_s[b]
        # (o=128c+p, m) -> ea[p, 64c+m]
        ea = np.ascontiguousarray(
            efp.reshape(NC, 128, NM).transpose(1, 0, 2).reshape(128, SW)
        ).astype(BF16)
        hsT = np.zeros((KS, NOP), BF16)
        hsT[:, 0:NO] = h_src[b].T
        in_maps.append({"hsT": hsT, "ea": ea, "par": parb})
    return in_maps


def kernel(**inputs):
    if "ope_ma_adj_batch" in inputs and "adj" not in inputs:
        inputs = dict(inputs)
        inputs["adj"] = inputs.pop("ope_ma_adj_batch")
    args = {k: np.asarray(inputs[k]) for k in
            ("h_src", "h_dst", "edge_feat", "adj", "W_src", "W_dst", "W_edge",
             "attn_l", "attn_r")}

    from concourse.bass_utils import run_bass_kernel_spmd

    nc = _get_nc()
    in_maps = _prep_core_inputs(**args)
    res = run_bass_kernel_spmd(nc, in_maps, core_ids=list(range(N_CORES)))
    out = np.stack([res.results[b]["out"] for b in range(B)], axis=0)
    return out.astype(np.float32)


# revision 41
# speedup vs baseline: 1.0104x; 1.0104x over previous
"""GATedge kernel for Trainium2, 8 NeuronCores, batch-parallel (1 batch elem / core).

v2: o-on-partitions layout - transpose-free.

Math (per batch b), restructured from the reference:
  s_e  = dot(W_edge, attn_l); wl = W_src @ attn_l; wr = W_dst @ attn_r  (host)
  score[o,m] = el[o] + er[m] + efm[o,m]   (adj multiply dropped: masked
    slots carry -1e30 in efm so their pre-mask value is irrelevant)
  p    = exp(max(score, 0.2*score))       (lrelu via max)
  pk   = exp(lrelu(2*er));  Z[m] = sum_o p + pk
  F^T  = sum_c fs_c^T @ p_c  (fs = h_src @ W_src)   [PE, o-contraction]
  S1   = g^T @ 1, g = p * efm;  Zcol = p^T @ 1      [PE ones-matmuls]
  out  = sigmoid((S1*W_edge/s_e + F + pk*feat_dst) / Z)

Layout: o padded to 1024 = 8 chunks x 128 partitions. Score tensors are
(128, 512): block c cols [64c,64c+64) hold o-chunk c (partition p = o-128c).
el enters the score PSUM via matmuls with lhsT=hsT_c, rhs=wl tiled 64 wide
(broadcast across m); er via lhsT=wr tiled 128 wide, rhs=hdT (broadcast
across partitions). Pad rows carry efm=-1e30 so p=0 there.
F^T (d-part, m-free) needs ONE transpose back to m-part for the combine.

All large tensors bf16 (rel err 1.3e-2 vs the 2e-2 gate); PSUM fp32.

Hardware rules learned the hard way (violations crash the exec unit):
 - never read PSUM rows a matmul didn't write
 - a PE transpose must be followed by its PSUM->SBUF copy before the next
   transpose into the same bank
 - tensor_tensor with both SBUF inputs needs equal base partitions
 - ACT Lrelu gives wrong results (alpha semantics) - use max(x, 0.2x)
"""

import sys

for _p in ("/opt/trn_rl_repo", "/root/.axon_site/_ro/trn_rl_repo"):
    if _p not in sys.path:
        sys.path.insert(0, _p)

import numpy as np
import ml_dtypes

BF16 = ml_dtypes.bfloat16

B, NO, NM, KS, KD, D = 8, 1000, 64, 128, 64, 128
NOP = 1024                # o padded
NC = 8                    # o-chunks
SW = NC * NM              # score width = 512
N_CORES = 8
NEG_SLOPE = 0.2
BIG = 1e30

# packed param buffer column layout (par0 = cols [0, P_WD); par1 = rest)
P_WL, P_WS, P_WR, P_HD, P_WC, P_ON, P_ID, P_OR = 0, 64, 192, 320, 384, 385, 386, 514
P_WD, P_WEB = 642, 770
P_COLS = 898

_cache = {}


def _build():
    import os
    import concourse.tile as tile
    from concourse import bacc, mybir
    from concourse.tile import add_dep_helper

    stage = int(os.environ.get("BASS_GAT_STAGE", "9"))
    f32 = mybir.dt.float32
    bf = mybir.dt.bfloat16
    AF = mybir.ActivationFunctionType
    ALU = mybir.AluOpType

    nc = bacc.Bacc("TRN2", target_bir_lowering=False, debug=False,
                   num_devices=N_CORES)

    d_hsT = nc.dram_tensor("hsT", [KS, NOP], bf, kind="ExternalInput")
    d_ea = nc.dram_tensor("ea", [128, SW], bf, kind="ExternalInput")
    d_par = nc.dram_tensor("par", [128, P_COLS], bf, kind="ExternalInput")
    d_out = nc.dram_tensor("out", [NM, D], f32, kind="ExternalOutput")

    def _emit(tc, sb, ps):
        # ---- input DMAs: par0(SP), hsT0(Pool swdge), ea0(ACT), hsT1(SP),
        # ea1(Pool swdge), par1(SP) ----
        s_par = sb.tile([128, P_COLS], bf, tag="par")
        nc.sync.dma_start(s_par[:, 0:P_WD], d_par[:, 0:P_WD])
        s_hsT = sb.tile([KS, NOP], bf, tag="hsT")
        nc.gpsimd.dma_start(s_hsT[:, 0:512], d_hsT[:, 0:512])
        s_ea = sb.tile([128, SW], bf, tag="ea")
        nc.scalar.dma_start(s_ea[:, 0:256], d_ea[:, 0:256])
        nc.sync.dma_start(s_hsT[:, 512:NOP], d_hsT[:, 512:NOP])
        nc.gpsimd.dma_start(s_ea[:, 256:SW], d_ea[:, 256:SW])
        nc.sync.dma_start(s_par[:, P_WD:P_COLS], d_par[:, P_WD:P_COLS])

        wl64 = s_par[:, P_WL:P_WL + NM]
        wsrc = s_par[:, P_WS:P_WS + D]
        wr128 = s_par[0:KD, P_WR:P_WR + D]
        hdT = s_par[0:KD, P_HD:P_HD + NM]
        wrcol = s_par[0:KD, P_WC:P_WC + 1]
        s_ones = s_par[:, P_ON:P_ON + 1]
        s_ident = s_par[:, P_ID:P_ID + D]   # host-provided 128x128 identity
        ones_row = s_par[0:1, P_OR:P_OR + D]
        wdst = s_par[0:KD, P_WD:P_WD + D]
        web128 = s_par[:, P_WEB:P_WEB + D]

        # ---- PE: er column for ekk; per chunk er+el accumulate pair
        # (er start=True must be immediately followed by its el start=False:
        # an intervening start=True on the same PSUM bank resets the group).
        # score/fs psum split per half: Tile deps are tile-granular, so one
        # big tile would gate chunk-0 consumers on chunk-7 producers.
        # ps_small regions: col0 er_col, col1 Z_col, cols 4:68 er_row (p0),
        # cols 68:132 S1_row (p0). Groups stay sequential within this bank.
        ps_small = ps.tile([KD, 132], f32, tag="small", padded_shape=[KD, 512])
        nc.tensor.matmul(ps_small[:, 0:1], hdT, wrcol, start=True, stop=True)
        nc.tensor.matmul(ps_small[0:1, 4:68], wrcol, hdT,
                         start=True, stop=True)

        # per block: [er, el, ea] accumulate triple (must stay adjacent: a
        # later start=True in the same PSUM bank resets open groups), ea via
        # identity-matmul so the full score lands in PSUM with no DVE add.
        ps_score = [ps.tile([128, SW // 2], f32, tag=f"score{i}",
                            name=f"ps_score{i}") for i in range(2)]
        ps_fs = [ps.tile([128, NC * D // 2], f32, tag=f"fs{i}",
                         name=f"ps_fs{i}") for i in range(2)]
        for i in range(2):
            for cc in range(4):
                c = 4 * i + cc
                h_c = s_hsT[:, D * c:D * c + D]
                blk = ps_score[i][:, NM * cc:NM * cc + NM]
                nc.tensor.matmul(blk, wr128, hdT, start=True, stop=False)
                nc.tensor.matmul(blk, h_c, wl64, start=False, stop=False)
                nc.tensor.matmul(blk, s_ident, s_ea[:, NM * c:NM * c + NM],
                                 start=False, stop=True)
            for cc in range(4):
                c = 4 * i + cc
                h_c = s_hsT[:, D * c:D * c + D]
                nc.tensor.matmul(ps_fs[i][:, D * cc:D * cc + D], h_c, wsrc,
                                 start=True, stop=True)
        if stage <= 1:
            s_dbg = sb.tile([NM, D], f32, tag="dbg")
            nc.vector.tensor_copy(s_dbg[:], ps_score[0][0:NM, 0:D])
            nc.sync.dma_start(d_out[:], s_dbg[:])
            return

        # ---- ekk/pk prologue, both orientations - off critical path ----
        # column form (for Z fold): 2*er -> lrelu(on Pool) -> exp
        s_er2 = sb.tile([64, 1], f32, tag="er2")
        nc.scalar.mul(s_er2[:], ps_small[:, 0:1], 2.0)
        s_er2s = sb.tile([64, 1], f32, tag="er2s")
        nc.vector.tensor_scalar_mul(s_er2s[:], s_er2[:], NEG_SLOPE)
        s_ekk = sb.tile([64, 1], f32, tag="ekk")
        r_ekk = nc.vector.tensor_max(s_ekk[:], s_er2[:], s_er2s[:])
        s_pk = sb.tile([64, 1], f32, tag="pk")
        r_pk = nc.scalar.activation(s_pk[:], s_ekk[:], AF.Exp)
        # row form (for the pk*fd^T term): prelu(2*er_row) -> exp -> bf16
        s_ekr = sb.tile([1, NM], f32, tag="ekr")
        nc.scalar.activation(s_ekr[:], ps_small[0:1, 4:68], AF.Prelu,
                             scale=2.0, alpha=NEG_SLOPE)
        s_pkr = sb.tile([1, NM], bf, tag="pkr")
        r_pkr = nc.scalar.activation(s_pkr[:], s_ekr[:], AF.Exp)
        # pk broadcast to 64 partitions, then scale hdT columns by pk
        ps_pkb = ps.tile([KD, NM], f32, tag="pkb", padded_shape=[KD, 512])
        nc.tensor.matmul(ps_pkb[:], ones_row[0:1, 0:KD], s_pkr[:],
                         start=True, stop=True)
        s_hdTs = sb.tile([KD, NM], bf, tag="hdTs")
        r_hdTs = nc.vector.tensor_tensor(s_hdTs[:], hdT, ps_pkb[:], ALU.mult)

        # ---- score chain, 2 column chunks of 256: lrelu then exp.
        # prelu=1: single ACT Prelu from PSUM; else DVE mul+max fallback.
        prelu = os.environ.get("BASS_GAT_PRELU", "1") == "1"
        s_fs = sb.tile([128, NC * D], bf, tag="fs_sb")
        s_ay = sb.tile([128, SW], bf, tag="ay")
        s_eijk = sb.tile([128, SW], bf, tag="eijk")
        s_p = sb.tile([128, SW], bf, tag="p")
        r_exp = [None, None]
        r_fs = [None, None]
        r_last = None
        for i in range(2):
            c = slice(256 * i, 256 * i + 256)
            if prelu:
                prev = r_last
                r_last = nc.scalar.activation(s_eijk[:, c], ps_score[i][:],
                                              AF.Prelu, alpha=NEG_SLOPE)
                if i == 0:
                    # keep the tiny pk/pkr ACT ops ahead of the score chain
                    add_dep_helper(r_last.ins, r_pk.ins,
                                   reason="pk before score chain on ACT")
                    add_dep_helper(r_last.ins, r_pkr.ins,
                                   reason="pkr before score chain on ACT")
                else:
                    add_dep_helper(r_last.ins, prev.ins,
                                   reason="score chunk order on ACT")
            else:
                r_a = nc.vector.tensor_scalar_mul(s_ay[:, c], ps_score[i][:],
                                                  NEG_SLOPE)
                if r_last is not None:
                    add_dep_helper(r_a.ins, r_last.ins,
                                   reason="score chain chunk order on DVE")
                r_last = nc.vector.tensor_max(s_eijk[:, c], ps_score[i][:],
                                              s_ay[:, c])
            r_exp[i] = nc.scalar.activation(s_p[:, c], s_eijk[:, c], AF.Exp)
            # fs copies on DVE so the ACT queue stays clear for prelu/exp
            r_fs[i] = nc.vector.tensor_copy(s_fs[:, 512 * i:512 * i + 512],
                                            ps_fs[i][:])
        # DVE order: er2s/ekk, fs_h1, hdTs, g_0, fs_h2, g_1
        add_dep_helper(r_fs[0].ins, r_ekk.ins, reason="ekk before fs_h1")
        add_dep_helper(r_hdTs.ins, r_fs[0].ins, reason="hdTs after fs_h1")

        def after_chain(r):
            add_dep_helper(r.ins, r_last.ins,
                           reason="defer small DVE op past score chain")
            return r

        if stage <= 3:
            s_dbg = sb.tile([NM, D], f32, tag="dbg")
            nc.vector.tensor_copy(s_dbg[:], s_p[0:NM, 0:D])
            nc.sync.dma_start(d_out[:], s_dbg[:])
            return

        # g = p * efm (for S1), per chunk on DVE right after each exp
        s_g = sb.tile([128, SW], bf, tag="g")
        r_g = [None, None]
        for i in range(2):
            c = slice(256 * i, 256 * i + 256)
            r_g[i] = nc.vector.tensor_tensor(s_g[:, c], s_p[:, c],
                                             s_ea[:, c], ALU.mult)
        add_dep_helper(r_fs[1].ins, r_hdTs.ins, reason="fs_h2 after hdTs")
        add_dep_helper(r_g[0].ins, r_fs[1].ins, reason="g_0 after fs_h2")
        add_dep_helper(r_g[1].ins, r_g[0].ins, reason="g chunk order")

        # ---- PE: the whole pre-sigmoid combine accumulates in ONE psum
        # tile tT (d-part, m-free): pk*fd^T (start) + F^T + web (x) S1 via
        # web128 x g_c matmuls (stop on the last). Z runs in ps_small after.
        ps_tT = ps.tile([D, NM], f32, tag="tT", padded_shape=[D, 512])
        nc.tensor.matmul(ps_tT[:], wdst, s_hdTs[:], start=True, stop=False)
        for c in range(NC):
            nc.tensor.matmul(ps_tT[:], s_fs[:, D * c:D * c + D],
                             s_p[:, NM * c:NM * c + NM],
                             start=False, stop=False)
        for c in range(NC):
            nc.tensor.matmul(ps_tT[:], web128, s_g[:, NM * c:NM * c + NM],
                             start=False, stop=(c == NC - 1))
        for c in range(NC):
            nc.tensor.matmul(ps_small[:, 1:2], s_p[:, NM * c:NM * c + NM],
                             s_ones[:], start=(c == 0), stop=(c == NC - 1))

        # Z = Zcol + pk; hZr = 0.5/Z (tanh scale)
        s_Z = sb.tile([64, 1], f32, tag="Z")
        after_chain(nc.vector.tensor_tensor(s_Z[:], ps_small[:, 1:2],
                                            s_pk[:], ALU.add))
        s_Zr = sb.tile([64, 1], f32, tag="Zr")
        after_chain(nc.vector.reciprocal(s_Zr[:], s_Z[:]))
        s_hZr = sb.tile([64, 1], f32, tag="hZr")
        after_chain(nc.vector.tensor_scalar_mul(s_hZr[:], s_Zr[:], 0.5))

        if stage <= 4:
            s_dbg = sb.tile([NM, D], f32, tag="dbg")
            nc.vector.tensor_scalar(s_dbg[:], s_p[0:NM, 0:D], s_Z[:],
                                    None, ALU.mult)
            nc.sync.dma_start(d_out[:], s_dbg[:])
            return

        # ---- tT -> (m, d) via one transpose, then sigmoid via tanh:
        # sigmoid(x) = 0.5 + 0.5*tanh(x/2)  (Tanh shares the Exp table) ----
        s_tT = sb.tile([D, NM], bf, tag="tT_sb")
        after_chain(nc.vector.tensor_copy(s_tT[:], ps_tT[:]))
        ps_F = ps.tile([NM, D], bf, tag="F", padded_shape=[NM, 512])
        nc.tensor.transpose(ps_F[:], s_tT[:], s_ident[:])
        s_th = sb.tile([NM, D], f32, tag="th")
        nc.scalar.activation(s_th[:], ps_F[:], AF.Tanh, scale=s_hZr[:])
        # prepared-SWDGE scatter output deadlocks TimelineSim's end barrier
        # (trigger completion track vs pool barrier SEQ cycle) - keep off
        scatter = os.environ.get("BASS_GAT_SCATTER", "0") == "1"
        if not scatter:
            s_out = sb.tile([NM, D], f32, tag="out_sb")
            after_chain(nc.vector.tensor_scalar(s_out[:], s_th[:], 0.5, 0.5,
                                                ALU.mult, ALU.add))
            nc.sync.dma_start(d_out[:], s_out[:])
            return
        # prepared-SWDGE output: descriptors are generated early off the
        # critical path; the trigger fires as soon as s_big is written,
        # skipping the HWDGE-gen + dge delay (~1.3us) of a plain dma_start.
        # d_out is zeroed first since scatter ADDs into DRAM.
        s_zero = sb.tile([NM, D], f32, tag="zero")
        nc.gpsimd.memset(s_zero[:], 0.0)
        nc.sync.dma_start(d_out[:], s_zero[:])
        s_idx = sb.tile([16, 4], mybir.dt.int16, tag="idx")
        nc.gpsimd.iota(s_idx[:], pattern=[[16, 4]], base=0,
                       channel_multiplier=1)
        s_big = sb.tile([128, 1, D], f32, tag="out_big")
        after_chain(nc.vector.tensor_scalar(s_big[0:NM, 0:1, :], s_th[:],
                                            0.5, 0.5, ALU.mult, ALU.add))
        dma_sem = nc.alloc_semaphore("out_dma")
        nc.gpsimd.dma_scatter_add(d_out[:], s_big[:], s_idx[:], NM, NM, D,
                                  prepare_only=True, sem=dma_sem)
        nc.gpsimd.trigger_dma(count=None)

    with tile.TileContext(nc) as tc:
        with tc.tile_pool(name="sb", bufs=1) as sb, \
             tc.tile_pool(name="ps", bufs=1, space="PSUM") as ps:
            _emit(tc, sb, ps)

    nc.compile()
    return nc


def _get_nc():
    if "nc" not in _cache:
        _cache["nc"] = _build()
    return _cache["nc"]


def _prep_core_inputs(h_src, h_dst, edge_feat, adj, W_src, W_dst, W_edge,
                      attn_l, attn_r):
    f32, f64 = np.float32, np.float64
    wl = (W_src.astype(f64) @ attn_l.astype(f64)).astype(f32)
    wr = (W_dst.astype(f64) @ attn_r.astype(f64)).astype(f32)
    s_e = float(np.dot(W_edge.astype(f64), attn_l.astype(f64)))
    s_e_safe = s_e if abs(s_e) > 1e-20 else 1e-20

    par = np.zeros((128, P_COLS), BF16)
    par[:, P_WL:P_WL + NM] = np.tile(wl[:, None], (1, NM))
    par[:, P_WS:P_WS + D] = W_src
    par[0:KD, P_WR:P_WR + D] = np.tile(wr[:, None], (1, D))
    par[0:KD, P_WC:P_WC + 1] = wr[:, None]
    par[:, P_ON:P_ON + 1] = 1.0
    par[:, P_ID:P_ID + D] = np.eye(128, dtype=f32)
    par[0:1, P_OR:P_OR + D] = 1.0
    par[0:KD, P_WD:P_WD + D] = W_dst
    par[:, P_WEB:P_WEB + D] = np.tile(
        (W_edge.astype(f64) / s_e_safe).astype(f32)[None, :], (128, 1))

    # fold the adjacency mask into the edge term: -1e30 where adj=0 makes
    # exp() zero those slots; pad rows (o >= 1000) also get -1e30
    ef_s = ((edge_feat.astype(f64) * s_e) +
            (adj.astype(f64) - 1.0) * BIG).astype(f32)  # (B, NO, NM)

    in_maps = []
    for b in range(B):
        parb = par.copy()
        parb[0:KD, P_HD:P_HD + NM] = h_dst[b].T
        efp = np.full((NOP, NM), -BIG, f32)
        efp[0:NO] = ef_s[b]
        # (o=128c+p, m) -> ea[p, 64c+m]
        ea = np.ascontiguousarray(
            efp.reshape(NC, 128, NM).transpose(1, 0, 2).reshape(128, SW)
        ).astype(BF16)
        hsT = np.zeros((KS, NOP), BF16)
        hsT[:, 0:NO] = h_src[b].T
        in_maps.append({"hsT": hsT, "ea": ea, "par": parb})
    return in_maps


def kernel(**inputs):
    if "ope_ma_adj_batch" in inputs and "adj" not in inputs:
        inputs = dict(inputs)
        inputs["adj"] = inputs.pop("ope_ma_adj_batch")
    args = {k: np.asarray(inputs[k]) for k in
            ("h_src", "h_dst", "edge_feat", "adj", "W_src", "W_dst", "W_edge",
             "attn_l", "attn_r")}

    from concourse.bass_utils import run_bass_kernel_spmd

    nc = _get_nc()
    in_maps = _prep_core_inputs(**args)
    res = run_bass_kernel_spmd(nc, in_maps, core_ids=list(range(N_CORES)))
    out = np.stack([res.results[b]["out"] for b in range(B)], axis=0)
    return out.astype(np.float32)
